# revision 17
# baseline (speedup 1.0000x reference)
"""CanonGLU feedforward layer on 8 TRN2 NeuronCores.

Math (per reference):
    gate = x @ w_gate.T ; up = x @ w_up.T            # [B,T,F]
    gate += causal_dconv(gate, conv_w[:F]) ; up += causal_dconv(up, conv_w[F:])
    out  = (up * silu(gate)) @ w_down.T              # [B,T,D]

Sharding: tensor-parallel over d_ff. Core c owns f-slice [c*1024,(c+1)*1024) of
w_gate/w_up/conv_w (column parallel) and w_down (row parallel); x replicated.
Each core computes a full-shape partial output; the host sums the 8 partials
(the "all-reduce").

Per-core layout: everything keeps d_ff (f) or d_model (d) on SBUF partitions
and tokens (t) on the free axis, so the depthwise conv is per-partition
scalar ops with shifted free-dim slices.  Matmul operands are fp16 (PE runs
fp16 at 1 cyc/row vs 4 for fp32; host-measured absmax/scale error 5e-4),
accumulation stays fp32 in PSUM.
"""

import numpy as np

import concourse.bass as bass
import concourse.mybir as mybir
import concourse.tile as tile
from concourse import bacc
from concourse.bass_utils import run_bass_kernel_spmd

F16 = mybir.dt.float16
F32 = mybir.dt.float32
AF = mybir.ActivationFunctionType
ALU = mybir.AluOpType

B, T, D, F = 2, 2048, 2048, 8192
NCORES = 8
FC_PER_CORE = F // NCORES          # 1024 f per core
TT = B * T                         # 4096 tokens total
NT = 512                           # token tile (one PSUM bank of fp32)
N_TILES = TT // NT                 # 8
TILES_PER_BATCH = T // NT          # 4 (conv halo resets at 0 and 4)
DC = D // 128                      # 16 d-chunks
FC = FC_PER_CORE // 128            # 8 f-chunks per core
GROW = NT + 4                      # conv buffer row: 3 halo + 512 data + 1 pad


def build_nc():
    nc = bacc.Bacc(None, target_bir_lowering=False, debug=False)

    xT = nc.dram_tensor("xT", [D, TT], F16, kind="ExternalInput")
    wgT = nc.dram_tensor("wgT", [D, FC_PER_CORE], F16, kind="ExternalInput")
    wuT = nc.dram_tensor("wuT", [D, FC_PER_CORE], F16, kind="ExternalInput")
    wdT = nc.dram_tensor("wdT", [FC_PER_CORE, D], F16, kind="ExternalInput")
    cw = nc.dram_tensor("cw", [128, FC, 2, 4], F32, kind="ExternalInput")
    outT = nc.dram_tensor("outT", [D, TT], F32, kind="ExternalOutput")

    xTr = xT.rearrange("(dc p) t -> p dc t", p=128)
    wgTr = wgT.rearrange("(dc p) f -> p dc f", p=128)
    wuTr = wuT.rearrange("(dc p) f -> p dc f", p=128)
    wdTr = wdT.rearrange("(fc p) d -> p fc d", p=128)

    with tile.TileContext(nc) as tc:
        with (
            tc.tile_pool(name="consts", bufs=1) as consts,
            tc.tile_pool(name="xp", bufs=2) as xpool,
            tc.tile_pool(name="gb", bufs=2) as gbpool,
            tc.tile_pool(name="ub", bufs=2) as ubpool,
            tc.tile_pool(name="hp", bufs=2 * FC) as hpool,
            tc.tile_pool(name="tp", bufs=6) as tpool,
            tc.tile_pool(name="op", bufs=3) as opool,
            tc.tile_pool(name="psg", bufs=2, space="PSUM") as ps_g,
            tc.tile_pool(name="psu", bufs=2, space="PSUM") as ps_u,
            tc.tile_pool(name="pso", bufs=4, space="PSUM") as ps_o,
        ):
            wg_sb = consts.tile([128, DC, FC_PER_CORE], F16)
            wu_sb = consts.tile([128, DC, FC_PER_CORE], F16)
            wd_sb = consts.tile([128, FC, D], F16)
            cw_sb = consts.tile([128, FC, 2, 4], F32)
            zero_b = consts.tile([128, 1], F32)
            nc.vector.memset(zero_b[:], 0.0)

            x_tiles = {}

            def load_x(tt, split=1):
                x_t = xpool.tile([128, DC, NT], F16)
                step = DC // split
                for i in range(split):
                    sl = slice(i * step, (i + 1) * step)
                    nc.sync.dma_start(
                        out=x_t[:, sl, :],
                        in_=xTr[:, sl, bass.ts(tt, NT)])
                x_tiles[tt] = x_t

            # DMA order matters at startup: x(0) first so the PE can start,
            # then gate/up weights per-chunk (matmuls start as chunks land),
            # conv weights, and w_down last (first needed ~54us in).
            # PE warmup: dummy matmuls on zeroed SBUF fill the startup DMA
            # wait so the HAM clock gate reaches (and keeps) 2.4 GHz before
            # the first real matmul.
            warm_sb = consts.tile([128, NT], F16)
            nc.vector.memset(warm_sb[:], 0.0)
            warm_ps = ps_g.tile([128, NT], F32, tag="ps")
            for _ in range(24):
                nc.tensor.matmul(
                    warm_ps[:], warm_sb[:, 0:128], warm_sb[:],
                    start=True, stop=True)

            load_x(0, split=8)
            for dc in range(DC):
                nc.sync.dma_start(out=wg_sb[:, dc, :], in_=wgTr[:, dc, :])
                nc.sync.dma_start(out=wu_sb[:, dc, :], in_=wuTr[:, dc, :])
            nc.sync.dma_start(out=cw_sb[:], in_=cw[:])
            for fc in range(FC):
                nc.sync.dma_start(out=wd_sb[:, fc, :], in_=wdTr[:, fc, :])

            g_prev = u_prev = None
            h_tiles = {}

            def gateup_phase(tt):
                nonlocal g_prev, u_prev
                x_t = x_tiles[tt]
                g_cur = gbpool.tile([128, FC, GROW], F16)
                u_cur = ubpool.tile([128, FC, GROW], F16)
                hs = []
                for br, (w_sb, buf, prev, psp) in enumerate((
                    (wg_sb, g_cur, g_prev, ps_g),
                    (wu_sb, u_cur, u_prev, ps_u),
                )):
                    # conv halo: last 3 tokens of the previous tile (zeros at
                    # the start of each batch — causal left pad).
                    if tt % TILES_PER_BATCH == 0:
                        nc.vector.memset(buf[:, :, 0:3], 0.0)
                    else:
                        nc.vector.tensor_copy(
                            out=buf[:, :, 0:3], in_=prev[:, :, NT:NT + 3])
                    for fc in range(FC):
                        ps = psp.tile([128, NT], F32)
                        for dc in range(DC):
                            nc.tensor.matmul(
                                ps[:],
                                w_sb[:, dc, bass.ts(fc, 128)],
                                x_t[:, dc, :],
                                start=(dc == 0),
                                stop=(dc == DC - 1),
                            )
                        nc.scalar.copy(out=buf[:, fc, 3:3 + NT], in_=ps[:])
                for fc in range(FC):
                    # causal depthwise conv + residual (folded into tap 3),
                    # then h = up_conv * silu(gate_conv).
                    conv_out = []
                    for br, buf in ((0, g_cur), (1, u_cur)):
                        t1 = tpool.tile([128, NT], F16)
                        # All taps on VectorE so ScalarE's stream stays
                        # homogeneous (Copy evacs, then Sigmoids) — avoids
                        # per-op activation-table reloads on HW.
                        nc.vector.tensor_scalar(
                            t1[:], buf[:, fc, 1:1 + NT],
                            cw_sb[:, fc, br, 1:2], None, ALU.mult)
                        for k in (0, 2, 3):
                            nc.vector.scalar_tensor_tensor(
                                out=t1[:], in0=buf[:, fc, k:k + NT],
                                scalar=cw_sb[:, fc, br, k:k + 1], in1=t1[:],
                                op0=ALU.mult, op1=ALU.add)
                        conv_out.append(t1)
                    gc, uc = conv_out
                    sg = tpool.tile([128, NT], F16)
                    nc.scalar.activation(
                        out=sg[:], in_=gc[:], func=AF.Sigmoid,
                        bias=zero_b[:, 0:1])
                    nc.vector.tensor_mul(sg[:], sg[:], gc[:])
                    h_t = hpool.tile([128, NT], F16)
                    nc.vector.tensor_mul(h_t[:], uc[:], sg[:])
                    hs.append(h_t)
                g_prev, u_prev = g_cur, u_cur
                h_tiles[tt] = hs

            def down_phase(tt):
                hs = h_tiles.pop(tt)
                for dc in range(DC):
                    pso = ps_o.tile([128, NT], F32)
                    for fc in range(FC):
                        nc.tensor.matmul(
                            pso[:],
                            wd_sb[:, fc, bass.ts(dc, 128)],
                            hs[fc][:],
                            start=(fc == 0),
                            stop=(fc == FC - 1),
                        )
                    o_sb = opool.tile([128, NT], F32)
                    nc.scalar.copy(out=o_sb[:], in_=pso[:])
                    nc.sync.dma_start(
                        out=outT[bass.ts(dc, 128), bass.ts(tt, NT)],
                        in_=o_sb[:])

            # Software pipeline: gate/up(tt) is emitted before down(tt-1) so
            # the PE never waits on the conv/act chain of the current tile.
            for tt in range(N_TILES + 1):
                if tt + 1 <= N_TILES - 1:
                    load_x(tt + 1)
                if tt < N_TILES:
                    gateup_phase(tt)
                if tt >= 1:
                    down_phase(tt - 1)

    nc.compile()
    return nc


_NC_CACHE = None


def _get_nc():
    global _NC_CACHE
    if _NC_CACHE is None:
        _NC_CACHE = build_nc()
    return _NC_CACHE


def _prep_inputs(x, w_gate, w_up, w_down, conv_w):
    xT = np.ascontiguousarray(
        x.reshape(TT, D).T).astype(np.float16)         # [D, TT]
    # conv weights: [2F, 4] -> per-core [128, FC, 2, 4], residual folded in
    cwf = conv_w.reshape(2, NCORES, FC, 128, 4).astype(np.float32)
    in_maps = []
    for c in range(NCORES):
        fs = slice(c * FC_PER_CORE, (c + 1) * FC_PER_CORE)
        wgT = np.ascontiguousarray(w_gate[fs].T).astype(np.float16)
        wuT = np.ascontiguousarray(w_up[fs].T).astype(np.float16)
        wdT = np.ascontiguousarray(w_down[:, fs].T).astype(np.float16)
        cwc = np.ascontiguousarray(
            cwf[:, c].transpose(2, 1, 0, 3))           # [128, FC, 2, 4]
        cwc[:, :, :, 3] += 1.0
        in_maps.append({"xT": xT, "wgT": wgT, "wuT": wuT, "wdT": wdT,
                        "cw": cwc})
    return in_maps


def run_spmd(in_maps, **kwargs):
    nc = _get_nc()
    return run_bass_kernel_spmd(
        nc, in_maps, core_ids=list(range(NCORES)), **kwargs)


def kernel(x, w_gate, w_up, w_down, conv_w):
    in_maps = _prep_inputs(
        np.asarray(x, dtype=np.float32), np.asarray(w_gate, dtype=np.float32),
        np.asarray(w_up, dtype=np.float32),
        np.asarray(w_down, dtype=np.float32),
        np.asarray(conv_w, dtype=np.float32))
    res = run_spmd(in_maps)
    acc = np.zeros((D, TT), np.float32)
    for r in res.results:
        acc += r["outT"]
    return np.ascontiguousarray(acc.T).reshape(B, T, D)


# revision 20
# speedup vs baseline: 1.0057x; 1.0057x over previous
"""CanonGLU feedforward layer on 8 TRN2 NeuronCores.

Math (per reference):
    gate = x @ w_gate.T ; up = x @ w_up.T            # [B,T,F]
    gate += causal_dconv(gate, conv_w[:F]) ; up += causal_dconv(up, conv_w[F:])
    out  = (up * silu(gate)) @ w_down.T              # [B,T,D]

Sharding: tensor-parallel over d_ff. Core c owns f-slice [c*1024,(c+1)*1024) of
w_gate/w_up/conv_w (column parallel) and w_down (row parallel); x replicated.
Each core computes a full-shape partial output; the host sums the 8 partials
(the "all-reduce").

Per-core layout: everything keeps d_ff (f) or d_model (d) on SBUF partitions
and tokens (t) on the free axis, so the depthwise conv is per-partition
scalar ops with shifted free-dim slices.  Matmul operands are fp16 (PE runs
fp16 at 1 cyc/row vs 4 for fp32; host-measured absmax/scale error 5e-4),
accumulation stays fp32 in PSUM.
"""

import numpy as np

import concourse.bass as bass
import concourse.mybir as mybir
import concourse.tile as tile
from concourse import bacc
from concourse.bass_utils import run_bass_kernel_spmd

F16 = mybir.dt.float16
F32 = mybir.dt.float32
AF = mybir.ActivationFunctionType
ALU = mybir.AluOpType

B, T, D, F = 2, 2048, 2048, 8192
NCORES = 8
FC_PER_CORE = F // NCORES          # 1024 f per core
TT = B * T                         # 4096 tokens total
NT = 512                           # token tile (one PSUM bank of fp32)
N_TILES = TT // NT                 # 8
TILES_PER_BATCH = T // NT          # 4 (conv halo resets at 0 and 4)
DC = D // 128                      # 16 d-chunks
FC = FC_PER_CORE // 128            # 8 f-chunks per core
GROW = NT + 4                      # conv buffer row: 3 halo + 512 data + 1 pad


def build_nc():
    nc = bacc.Bacc(None, target_bir_lowering=False, debug=False)

    xT = nc.dram_tensor("xT", [D, TT], F16, kind="ExternalInput")
    wgT = nc.dram_tensor("wgT", [D, FC_PER_CORE], F16, kind="ExternalInput")
    wuT = nc.dram_tensor("wuT", [D, FC_PER_CORE], F16, kind="ExternalInput")
    wdT = nc.dram_tensor("wdT", [FC_PER_CORE, D], F16, kind="ExternalInput")
    cw = nc.dram_tensor("cw", [128, FC, 2, 4], F32, kind="ExternalInput")
    outT = nc.dram_tensor("outT", [D, TT], F32, kind="ExternalOutput")

    xTr = xT.rearrange("(dc p) t -> p dc t", p=128)
    wgTr = wgT.rearrange("(dc p) f -> p dc f", p=128)
    wuTr = wuT.rearrange("(dc p) f -> p dc f", p=128)
    wdTr = wdT.rearrange("(fc p) d -> p fc d", p=128)

    with tile.TileContext(nc) as tc:
        with (
            tc.tile_pool(name="consts", bufs=1) as consts,
            tc.tile_pool(name="xp", bufs=2) as xpool,
            tc.tile_pool(name="gb", bufs=2) as gbpool,
            tc.tile_pool(name="ub", bufs=2) as ubpool,
            tc.tile_pool(name="hp", bufs=2 * FC) as hpool,
            tc.tile_pool(name="tp", bufs=6) as tpool,
            tc.tile_pool(name="op", bufs=3) as opool,
            tc.tile_pool(name="psg", bufs=2, space="PSUM") as ps_g,
            tc.tile_pool(name="psu", bufs=2, space="PSUM") as ps_u,
            tc.tile_pool(name="pso", bufs=4, space="PSUM") as ps_o,
        ):
            wg_sb = consts.tile([128, DC, FC_PER_CORE], F16)
            wu_sb = consts.tile([128, DC, FC_PER_CORE], F16)
            wd_sb = consts.tile([128, FC, D], F16)
            cw_sb = consts.tile([128, FC, 2, 4], F32)
            zero_b = consts.tile([128, 1], F32)
            nc.vector.memset(zero_b[:], 0.0)

            x_tiles = {}

            def load_x(tt, split=1):
                x_t = xpool.tile([128, DC, NT], F16)
                step = DC // split
                for i in range(split):
                    sl = slice(i * step, (i + 1) * step)
                    nc.sync.dma_start(
                        out=x_t[:, sl, :],
                        in_=xTr[:, sl, bass.ts(tt, NT)])
                x_tiles[tt] = x_t

            # DMA order matters at startup: x(0) first so the PE can start,
            # then gate/up weights per-chunk (matmuls start as chunks land),
            # conv weights, and w_down last (first needed ~54us in).
            # PE warmup: dummy matmuls on zeroed SBUF fill the startup DMA
            # wait so the HAM clock gate reaches (and keeps) 2.4 GHz before
            # the first real matmul.
            warm_sb = consts.tile([128, NT], F16)
            nc.gpsimd.memset(warm_sb[:], 0.0)
            warm_ps = ps_g.tile([128, NT], F32, tag="ps")
            for _ in range(24):
                nc.tensor.matmul(
                    warm_ps[:], warm_sb[:, 0:128], warm_sb[:],
                    start=True, stop=True)

            load_x(0, split=8)
            # gate consumes w_gate first; all of w_gate before any w_up so the
            # first gate psum group isn't gated on the tail of the interleave.
            for dc in range(DC):
                nc.sync.dma_start(out=wg_sb[:, dc, :], in_=wgTr[:, dc, :])
            for dc in range(DC):
                nc.sync.dma_start(out=wu_sb[:, dc, :], in_=wuTr[:, dc, :])
            nc.sync.dma_start(out=cw_sb[:], in_=cw[:])
            for fc in range(FC):
                nc.sync.dma_start(out=wd_sb[:, fc, :], in_=wdTr[:, fc, :])

            g_prev = u_prev = None
            h_tiles = {}

            def gateup_phase(tt):
                nonlocal g_prev, u_prev
                x_t = x_tiles[tt]
                g_cur = gbpool.tile([128, FC, GROW], F16)
                u_cur = ubpool.tile([128, FC, GROW], F16)
                hs = []
                for br, (w_sb, buf, prev, psp) in enumerate((
                    (wg_sb, g_cur, g_prev, ps_g),
                    (wu_sb, u_cur, u_prev, ps_u),
                )):
                    # conv halo: last 3 tokens of the previous tile (zeros at
                    # the start of each batch — causal left pad).
                    if tt % TILES_PER_BATCH == 0:
                        nc.vector.memset(buf[:, :, 0:3], 0.0)
                    else:
                        nc.vector.tensor_copy(
                            out=buf[:, :, 0:3], in_=prev[:, :, NT:NT + 3])
                    for fc in range(FC):
                        ps = psp.tile([128, NT], F32)
                        for dc in range(DC):
                            nc.tensor.matmul(
                                ps[:],
                                w_sb[:, dc, bass.ts(fc, 128)],
                                x_t[:, dc, :],
                                start=(dc == 0),
                                stop=(dc == DC - 1),
                            )
                        nc.scalar.copy(out=buf[:, fc, 3:3 + NT], in_=ps[:])
                for fc in range(FC):
                    # causal depthwise conv + residual (folded into tap 3),
                    # then h = up_conv * silu(gate_conv).
                    conv_out = []
                    for br, buf in ((0, g_cur), (1, u_cur)):
                        t1 = tpool.tile([128, NT], F16)
                        # All taps on VectorE so ScalarE's stream stays
                        # homogeneous (Copy evacs, then Sigmoids) — avoids
                        # per-op activation-table reloads on HW.
                        nc.vector.tensor_scalar(
                            t1[:], buf[:, fc, 1:1 + NT],
                            cw_sb[:, fc, br, 1:2], None, ALU.mult)
                        for k in (0, 2, 3):
                            nc.vector.scalar_tensor_tensor(
                                out=t1[:], in0=buf[:, fc, k:k + NT],
                                scalar=cw_sb[:, fc, br, k:k + 1], in1=t1[:],
                                op0=ALU.mult, op1=ALU.add)
                        conv_out.append(t1)
                    gc, uc = conv_out
                    sg = tpool.tile([128, NT], F16)
                    nc.scalar.activation(
                        out=sg[:], in_=gc[:], func=AF.Sigmoid,
                        bias=zero_b[:, 0:1])
                    nc.vector.tensor_mul(sg[:], sg[:], gc[:])
                    h_t = hpool.tile([128, NT], F16)
                    nc.vector.tensor_mul(h_t[:], uc[:], sg[:])
                    hs.append(h_t)
                g_prev, u_prev = g_cur, u_cur
                h_tiles[tt] = hs

            def down_phase(tt):
                hs = h_tiles.pop(tt)
                for dc in range(DC):
                    pso = ps_o.tile([128, NT], F32)
                    for fc in range(FC):
                        nc.tensor.matmul(
                            pso[:],
                            wd_sb[:, fc, bass.ts(dc, 128)],
                            hs[fc][:],
                            start=(fc == 0),
                            stop=(fc == FC - 1),
                        )
                    o_sb = opool.tile([128, NT], F32)
                    nc.scalar.copy(out=o_sb[:], in_=pso[:])
                    # alternate the two HWDGE queues so the final tile's
                    # output drain is not serialized behind one queue
                    eng = nc.sync if dc % 2 == 0 else nc.scalar
                    eng.dma_start(
                        out=outT[bass.ts(dc, 128), bass.ts(tt, NT)],
                        in_=o_sb[:])

            # Software pipeline: gate/up(tt) is emitted before down(tt-1) so
            # the PE never waits on the conv/act chain of the current tile.
            for tt in range(N_TILES + 1):
                if tt + 1 <= N_TILES - 1:
                    load_x(tt + 1)
                if tt < N_TILES:
                    gateup_phase(tt)
                if tt >= 1:
                    down_phase(tt - 1)

    nc.compile()
    return nc


_NC_CACHE = None


def _get_nc():
    global _NC_CACHE
    if _NC_CACHE is None:
        _NC_CACHE = build_nc()
    return _NC_CACHE


def _prep_inputs(x, w_gate, w_up, w_down, conv_w):
    xT = np.ascontiguousarray(
        x.reshape(TT, D).T).astype(np.float16)         # [D, TT]
    # conv weights: [2F, 4] -> per-core [128, FC, 2, 4], residual folded in
    cwf = conv_w.reshape(2, NCORES, FC, 128, 4).astype(np.float32)
    in_maps = []
    for c in range(NCORES):
        fs = slice(c * FC_PER_CORE, (c + 1) * FC_PER_CORE)
        wgT = np.ascontiguousarray(w_gate[fs].T).astype(np.float16)
        wuT = np.ascontiguousarray(w_up[fs].T).astype(np.float16)
        wdT = np.ascontiguousarray(w_down[:, fs].T).astype(np.float16)
        cwc = np.ascontiguousarray(
            cwf[:, c].transpose(2, 1, 0, 3))           # [128, FC, 2, 4]
        cwc[:, :, :, 3] += 1.0
        in_maps.append({"xT": xT, "wgT": wgT, "wuT": wuT, "wdT": wdT,
                        "cw": cwc})
    return in_maps


def run_spmd(in_maps, **kwargs):
    nc = _get_nc()
    return run_bass_kernel_spmd(
        nc, in_maps, core_ids=list(range(NCORES)), **kwargs)


def kernel(x, w_gate, w_up, w_down, conv_w):
    in_maps = _prep_inputs(
        np.asarray(x, dtype=np.float32), np.asarray(w_gate, dtype=np.float32),
        np.asarray(w_up, dtype=np.float32),
        np.asarray(w_down, dtype=np.float32),
        np.asarray(conv_w, dtype=np.float32))
    res = run_spmd(in_maps)
    acc = np.zeros((D, TT), np.float32)
    for r in res.results:
        acc += r["outT"]
    return np.ascontiguousarray(acc.T).reshape(B, T, D)
